# revision 1
# baseline (speedup 1.0000x reference)
"""Squeeze-and-Excitation gate kernel for Trainium2 (Bass/Tile).

Reference computation (per sample b):
    s = mean(x[b], axis=(H, W))                # [C]
    h = relu(w1 @ s + b1)                      # [Cr]
    g = sigmoid(w2 @ h + b2)                   # [C]
    out[b] = x[b] * g[:, None, None]

Sharding: data-parallel over batch across 8 NeuronCores (8 samples each),
gate weights replicated. Each core streams each sample through SBUF once:
load -> pool (DVE reduce) -> tiny MLP (PE) -> sigmoid (ACT) -> scale (ACT)
-> store, so HBM traffic is the minimum 1x read + 1x write of x.
"""

import contextlib

import numpy as np

import concourse.bacc as bacc
import concourse.mybir as mybir
import concourse.tile as tile
from concourse import bass_utils
from concourse.masks import make_identity

N_CORES = 8
B, C, H, W = 64, 512, 56, 56
HW = H * W              # 3136
BL = B // N_CORES       # 8 local samples per core
P = 128                 # SBUF partitions
NCH = C // P            # 4 channel chunks of 128
R = 32                  # squeezed channels (Cr)
INV_HW = 1.0 / HW

_CACHE = {}
LAST_RESULTS = None     # test harness reads trace/exec info from here


def _emit(ctx, tc, out, x, w1, b1, w2, b2):
    nc = tc.nc
    f32 = mybir.dt.float32

    singles = ctx.enter_context(tc.tile_pool(name="singles", bufs=1))
    xpool = ctx.enter_context(tc.tile_pool(name="xpool", bufs=8))
    spool = ctx.enter_context(tc.tile_pool(name="spool", bufs=4))
    hpool = ctx.enter_context(tc.tile_pool(name="hpool", bufs=4))
    gpool = ctx.enter_context(tc.tile_pool(name="gpool", bufs=4))
    pp_h = ctx.enter_context(tc.tile_pool(name="pp_h", bufs=2, space="PSUM"))
    pp_g = ctx.enter_context(tc.tile_pool(name="pp_g", bufs=2, space="PSUM"))
    pp_t = ctx.enter_context(tc.tile_pool(name="pp_t", bufs=1, space="PSUM"))

    # ---- one-time setup: weights/biases into matmul-ready layouts ----
    ident = singles.tile([P, P], f32)
    make_identity(nc, ident)

    w1n = singles.tile([R, C], f32)                  # w1 natural [32, 512]
    nc.sync.dma_start(out=w1n, in_=w1)
    w2n = singles.tile([P, NCH, R], f32)             # w2n[p, t, r] = w2[t*128+p, r]
    nc.sync.dma_start(out=w2n, in_=w2.rearrange("(t p) r -> p t r", p=P))
    b1t = singles.tile([R, 1], f32)
    nc.sync.dma_start(out=b1t, in_=b1.rearrange("(r o) -> r o", o=1))
    b2t = singles.tile([P, NCH], f32)                # b2t[p, t] = b2[t*128+p]
    nc.sync.dma_start(out=b2t, in_=b2.rearrange("(t p) -> p t", p=P))

    # w1T[:, t, :] = w1[:, t*128:(t+1)*128].T * (1/HW)  -> lhsT for h-matmul
    # w2T[:, t, :] = w2[t*128:(t+1)*128, :].T           -> lhsT for g-matmul
    w1T = singles.tile([P, NCH, R], f32)
    w2T = singles.tile([R, NCH, P], f32)
    for t in range(NCH):
        ps1 = pp_t.tile([P, R], f32, tag="ps1")
        nc.tensor.transpose(ps1, w1n[:, t * P:(t + 1) * P], ident[:R, :R])
        nc.scalar.mul(w1T[:, t, :], ps1, INV_HW)
        ps2 = pp_t.tile([R, P], f32, tag="ps2")
        nc.tensor.transpose(ps2, w2n[:, t, :], ident)
        nc.scalar.copy(w2T[:, t, :], ps2)

    # ---- main loop over local samples ----
    for b in range(BL):
        s = spool.tile([P, NCH], f32)                # pooled sums per chunk
        xts = []
        for t in range(NCH):
            xt = xpool.tile([P, HW], f32, tag="x")
            nc.sync.dma_start(out=xt, in_=x[b, t * P:(t + 1) * P, :])
            nc.vector.reduce_sum(s[:, t:t + 1], xt, axis=mybir.AxisListType.X)
            xts.append(xt)

        # h = relu(w1 @ mean + b1): accumulate over the 4 channel chunks
        ph = pp_h.tile([R, 1], f32)
        for t in range(NCH):
            nc.tensor.matmul(ph, w1T[:, t, :], s[:, t:t + 1],
                             start=(t == 0), stop=(t == NCH - 1))
        h = hpool.tile([R, 1], f32)
        nc.vector.tensor_scalar(out=h, in0=ph, scalar1=b1t, scalar2=0.0,
                                op0=mybir.AluOpType.add, op1=mybir.AluOpType.max)

        # g[t] = sigmoid(w2[t] @ h + b2[t])
        pg = pp_g.tile([P, NCH], f32)
        g = gpool.tile([P, NCH], f32)
        for t in range(NCH):
            nc.tensor.matmul(pg[:, t:t + 1], w2T[:, t, :], h, start=True, stop=True)
            nc.scalar.activation(g[:, t:t + 1], pg[:, t:t + 1],
                                 mybir.ActivationFunctionType.Sigmoid,
                                 bias=b2t[:, t:t + 1], scale=1.0)

        # scale in place and store
        for t in range(NCH):
            nc.scalar.mul(xts[t], xts[t], g[:, t:t + 1])
            nc.sync.dma_start(out=out[b, t * P:(t + 1) * P, :], in_=xts[t])


def _build():
    f32 = mybir.dt.float32
    nc = bacc.Bacc("TRN2", target_bir_lowering=False, debug=False,
                   num_devices=N_CORES)
    x = nc.dram_tensor("x", [BL, C, HW], f32, kind="ExternalInput").ap()
    w1 = nc.dram_tensor("w1", [R, C], f32, kind="ExternalInput").ap()
    b1 = nc.dram_tensor("b1", [R], f32, kind="ExternalInput").ap()
    w2 = nc.dram_tensor("w2", [C, R], f32, kind="ExternalInput").ap()
    b2 = nc.dram_tensor("b2", [C], f32, kind="ExternalInput").ap()
    out = nc.dram_tensor("out", [BL, C, HW], f32, kind="ExternalOutput").ap()

    with tile.TileContext(nc) as tc:
        with contextlib.ExitStack() as ctx:
            _emit(ctx, tc, out, x, w1, b1, w2, b2)
    nc.compile()
    return nc


def _get_module():
    if "nc" not in _CACHE:
        _CACHE["nc"] = _build()
    return _CACHE["nc"]


def kernel(**inputs):
    global LAST_RESULTS
    x = np.ascontiguousarray(inputs["x"], dtype=np.float32)
    w1 = np.ascontiguousarray(inputs["w1"], dtype=np.float32)
    b1 = np.ascontiguousarray(inputs["b1"], dtype=np.float32)
    w2 = np.ascontiguousarray(inputs["w2"], dtype=np.float32)
    b2 = np.ascontiguousarray(inputs["b2"], dtype=np.float32)

    nc = _get_module()
    xr = x.reshape(B, C, HW)
    in_maps = [
        {
            "x": xr[i * BL:(i + 1) * BL],
            "w1": w1,
            "b1": b1,
            "w2": w2,
            "b2": b2,
        }
        for i in range(N_CORES)
    ]
    res = bass_utils.run_bass_kernel_spmd(
        nc, in_maps, core_ids=list(range(N_CORES))
    )
    LAST_RESULTS = res
    out = np.concatenate([res.results[i]["out"] for i in range(N_CORES)], axis=0)
    return out.reshape(B, C, H, W)

